# revision 1
# baseline (speedup 1.0000x reference)
"""Trainium2 Bass kernel for a basis-customized linear layer.

Reference computation (B=1024, IN=OUT=512, EMB=64, KQ=64, NB=3, VOCAB=100):
    embs = concat(emb_author[idx_author], emb_citation[idx_citation])  # [B, 128]
    h    = tanh(embs @ W1.T + b1)                                      # [B, 64]
    coef = softmax(h @ W2.T)                                           # [B, 3]
    w    = (coef @ W3.T + b3).reshape(B, IN, OUT)
    out  = einsum('bi,bio->bo', x, w)                                  # [B, 512]

Key rewrites:
  (1) w[b] = sum_j coef[b,j]*W3j + b3r and softmax coefs sum to 1, so
      out = sum_j coef[:,j] * (x @ (W3j + b3r))
      -- 3 shared [512,512] matmuls + a per-sample weighted combine.
  (2) the embedding gather is a one-hot matmul (one-hot from a DMA'd idx row
      compared against an iota via a K=2 matmul), fused with W1 via the
      host-precomputed per-vocab table G = emb @ W1half.T (+ b1/2 folded in).
  (3) x / W3 / gather tables / output are bf16 (half the DMA bytes on a
      ~250 GB/s-per-core HBM path); accumulation stays f32 in PSUM.

Sharding over 8 cores: batch 4-way x out-column 2-way. Each core holds
x.T for its 256 batch rows (bf16, 256KB) and its 256 out-columns of all
3 bases (bf16, 786KB), computes coef for its rows on-device, and writes a
[256, 256] output block.

Scheduling notes:
  - Per-core HBM bandwidth (~250 GB/s) is the wall: bytes stream in
    consumption order, alternating the two HWDGE rings.
  - Dummy warm-up matmuls keep the PE activity monitor from throttling;
    once the PE stays busy ~3us its clock doubles (384-row matmul cadence
    drops 632ns -> 195ns).
  - The softmax chain's matmuls are force-interleaved into the main sweep
    (add_dep_helper) so coef is ready when the first strip finishes.
  - The per-sample combine is split across engines: batch-tile 0 on DVE
    (mult+reduce), batch-tile 1 on ACT (per-partition-scaled copies per
    basis) + GpSimd (adds), so the tail after the last matmul is short.
"""

import numpy as np
import ml_dtypes

import concourse.bass as bass
import concourse.tile as tile
from concourse import bacc, mybir
from concourse.bass_utils import run_bass_kernel_spmd
from concourse.tile_rust import add_dep_helper

# Problem dims (hardcoded per contract)
B, IN, OUT = 1024, 512, 512
EMB, KQ, NB, VOCAB = 64, 64, 3, 100
P_B, Q_O = 4, 2            # batch shards x out-col shards = 8 cores
BS = B // P_B              # 256 batch rows per core
OB = 2                     # out-col strips per core
OSL = [160, 96]            # asymmetric strip widths: big strip streams
                           # first, small strip keeps the tail short
OOFF = [0, OSL[0]]         # strip col offsets within the 256-col shard
SW3 = [w * NB for w in OSL]        # strip matmul widths (<=512 psum bank)
SOFF = [0, (IN // 128) * SW3[0]]   # strip offsets in the flat wc tensor
KT = IN // 128             # 4 contraction tiles
MT = BS // 128             # 2 batch tiles per core
WCC = KT * (SW3[0] + SW3[1])       # total wc cols
OBASE = [0, MT * OSL[0]]   # strip offsets in the flat out tensor

F32 = mybir.dt.float32
BF16 = mybir.dt.bfloat16

G2 = 2 * KQ                # gather-table columns
IC0 = G2 + 4               # idx block starts here (after W2.T | pad)
IR0 = 64                   # idx/iota rows (matmul base partition: 0/32/64)
HC = IC0 + 2 * BS + VOCAB  # header cols: tables | W2 | idx | iota

LAST_RESULT = None         # BassKernelResults of the most recent run (for test.py)

_NC_CACHE = None


def _ensure_ntff_hook_module():
    """bass_utils imports antenv.axon_hooks when BASS_TRACE is set; the module
    is absent on this image. Provide a no-op shim so tracing degrades
    gracefully instead of crashing."""
    import sys, types
    if "antenv.axon_hooks" in sys.modules:
        return
    try:
        import antenv
        import antenv.axon_hooks  # noqa: F401
    except ImportError:
        mod = types.ModuleType("antenv.axon_hooks")
        state = {"hook": None}
        mod.set_axon_ntff_profile_hook = lambda h: state.__setitem__("hook", h)
        mod.get_axon_ntff_profile_hook = lambda: state["hook"]
        sys.modules["antenv.axon_hooks"] = mod
        try:
            antenv.axon_hooks = mod
        except Exception:
            pass


def _bcast_os(ap_2d, width):
    """[128, N] AP -> [128, width, N] AP with a stride-0 middle dim."""
    return bass.AP(
        tensor=ap_2d.tensor, offset=ap_2d.offset,
        ap=[list(ap_2d.ap[0]), [0, width], list(ap_2d.ap[1])],
    )


def _build_nc():
    nc = bacc.Bacc("TRN2", target_bir_lowering=False, debug=False,
                   num_devices=P_B * Q_O)

    hdr = nc.dram_tensor("hdr", [128, HC], BF16, kind="ExternalInput")
    xt = nc.dram_tensor("xt", [128, KT * BS], BF16, kind="ExternalInput")
    wc = nc.dram_tensor("wc", [128, WCC], BF16, kind="ExternalInput")
    out = nc.dram_tensor("out", [128, MT * (OSL[0] + OSL[1])], BF16,
                         kind="ExternalOutput")

    with tile.TileContext(nc) as tc:
        with (
            tc.tile_pool(name="consts", bufs=1) as consts,
            tc.tile_pool(name="work", bufs=4) as work,
            tc.tile_pool(name="ps_idx", bufs=1, space="PSUM") as ps_idx,
            tc.tile_pool(name="ps_pre", bufs=1, space="PSUM") as ps_pre,
            tc.tile_pool(name="ps_y", bufs=1, space="PSUM") as ps_y,
        ):
            # ---- loads, consumption order on the two HWDGE rings
            hdr_sb = consts.tile([128, HC], BF16)
            nc.sync.dma_start(out=hdr_sb, in_=hdr[:, :])
            xall = consts.tile([128, KT, BS], BF16)
            nc.scalar.dma_start(out=xall, in_=xt[:, :].rearrange(
                "p (k n) -> p k n", k=KT))
            wall = consts.tile([128, WCC], BF16)
            nc.sync.dma_start(out=wall[:, 0:SOFF[1]], in_=wc[:, 0:SOFF[1]])
            nc.scalar.dma_start(out=wall[:, SOFF[1]:WCC],
                                in_=wc[:, SOFF[1]:WCC])

            gat_sb = hdr_sb[0:VOCAB, 0:KQ]
            gct_sb = hdr_sb[0:VOCAB, KQ:G2]
            w2r_sb = hdr_sb[0:KQ, G2:G2 + NB + 1]
            idx_sb = hdr_sb[IR0:IR0 + 2, IC0:IC0 + 2 * BS]
            bw_sb = hdr_sb[IR0:IR0 + 2, IC0 + 2 * BS:IC0 + 2 * BS + VOCAB]

            coefc = consts.tile([128, MT, NB], F32)
            with tc.high_priority():
                # PE warm-up on a zeroed tile: ramps the PE clock before the
                # real sweep (and keeps it busy while waiting on DMAs).
                junk = consts.tile([128, 512], BF16)
                nc.vector.memset(junk.bitcast(mybir.dt.uint32), 0)
                wm_ps = ps_idx.tile([128, 512], F32, tag="warm")
                for w in range(7):
                    nc.tensor.matmul(wm_ps, lhsT=junk[:, 0:128], rhs=junk,
                                     start=True, stop=True)

                # ---- stage A: coef for all BS rows ----
                # one-hot via K=2 matmul: psum[v,b] = idx[b] - v, then ==0
                bc_ps = ps_idx.tile([VOCAB, 2 * BS], F32, tag="idx")
                nc.tensor.matmul(bc_ps, lhsT=bw_sb, rhs=idx_sb, start=True,
                                 stop=True)
                # keep the PE clock ramping while DVE runs is_equal
                warm2 = []
                for w in range(3):
                    warm2.append(nc.tensor.matmul(
                        wm_ps, lhsT=junk[:, 0:128], rhs=junk,
                        start=True, stop=True))
                # per-table is_equal halves so the author gather can start
                # before the citation one-hot is finished
                oh_sb = consts.tile([VOCAB, 2 * BS], BF16)
                nc.vector.tensor_scalar(
                    out=oh_sb[:, 0:BS], in0=bc_ps[:, 0:BS], scalar1=0.0,
                    scalar2=None, op0=mybir.AluOpType.is_equal,
                )
                nc.vector.tensor_scalar(
                    out=oh_sb[:, BS:2 * BS], in0=bc_ps[:, BS:2 * BS],
                    scalar1=0.0, scalar2=None,
                    op0=mybir.AluOpType.is_equal,
                )

                # fused gather + W1 (+b1): preact.T [KQ, BS]
                pre_ps = ps_pre.tile([KQ, BS], F32, tag="pre")
                g1 = nc.tensor.matmul(pre_ps, lhsT=gat_sb,
                                      rhs=oh_sb[:, 0:BS],
                                      start=True, stop=False)
                g2 = nc.tensor.matmul(pre_ps, lhsT=gct_sb,
                                      rhs=oh_sb[:, BS:2 * BS],
                                      start=False, stop=True)
                ht_sb = consts.tile([KQ, BS], BF16)
                nc.scalar.activation(
                    out=ht_sb, in_=pre_ps,
                    func=mybir.ActivationFunctionType.Tanh,
                )

                # logits for both batch tiles into one PSUM bank
                lgall = ps_pre.tile([128, MT, NB + 1], F32, tag="lg")
                lmms = []
                for m in range(MT):
                    lmms.append(nc.tensor.matmul(
                        lgall[:, m, :], lhsT=ht_sb[:, m * 128:(m + 1) * 128],
                        rhs=w2r_sb, start=True, stop=True,
                    ))
                # merged softmax epilogue: one exp, one reduce, one recip,
                # one broadcast multiply
                e_sb = work.tile([128, MT, NB], F32, tag="e")
                nc.scalar.activation(
                    out=e_sb, in_=lgall[:, :, 0:NB],
                    func=mybir.ActivationFunctionType.Exp,
                )
                s_sb = work.tile([128, MT], F32, tag="s")
                nc.vector.reduce_sum(out=s_sb, in_=e_sb,
                                     axis=mybir.AxisListType.X)
                r_sb = work.tile([128, MT], F32, tag="r")
                nc.vector.reciprocal(out=r_sb, in_=s_sb)
                rb = bass.AP(
                    tensor=r_sb.tensor, offset=r_sb.offset,
                    ap=[list(r_sb.ap[0]), list(r_sb.ap[1]), [0, NB]],
                )
                nc.vector.tensor_tensor(out=coefc, in0=e_sb, in1=rb,
                                        op=mybir.AluOpType.mult)

            # ---- stage B: per strip ob: Y[m][b, o, j] over k; combine
            # m0 on DVE, m1 on ACT+GpSimd; store each block as it's done.
            out_sb = consts.tile([128, MT * (OSL[0] + OSL[1])], BF16)
            first_mm = {}
            for ob in range(OB):
                w = OSL[ob]
                y_ps = ps_y.tile([128, MT, 512], F32, name=f"y{ob}",
                                 tag=f"y{ob}")
                for k in range(KT):
                    for m in range(MT):
                        mm = nc.tensor.matmul(
                            y_ps[:, m, 0:SW3[ob]].rearrange(
                                "p (o j) -> p o j", j=NB),
                            lhsT=xall[:, k, m * 128:(m + 1) * 128],
                            rhs=wall[:, SOFF[ob] + k * SW3[ob]:
                                     SOFF[ob] + (k + 1) * SW3[ob]].rearrange(
                                "p (o j) -> p o j", j=NB),
                            start=(k == 0), stop=(k == KT - 1),
                        )
                        first_mm[(ob, k, m)] = mm

                yv = y_ps[:, :, 0:SW3[ob]].rearrange(
                    "p m (o j) -> p m o j", j=NB)
                ocols = [slice(OBASE[ob] + m * w, OBASE[ob] + (m + 1) * w)
                         for m in range(MT)]
                if ob == 0:
                    # m0 on DVE (mult+reduce); m1 via ACT per-basis scaled
                    # copies + GpSimd adds: all overlap strip 1's matmuls
                    tmp = work.tile([128, OSL[0], NB], BF16, tag="tmp")
                    nc.vector.tensor_tensor(
                        out=tmp[:, 0:w], in0=yv[:, 0],
                        in1=_bcast_os(coefc[:, 0, :], w),
                        op=mybir.AluOpType.mult,
                    )
                    with nc.allow_low_precision("bf16 out, tol 2e-2"):
                        nc.vector.reduce_sum(
                            out=out_sb[:, ocols[0]], in_=tmp[:, 0:w],
                            axis=mybir.AxisListType.X,
                        )
                    nc.sync.dma_start(out=out[:, ocols[0]],
                                      in_=out_sb[:, ocols[0]])
                    tj = [work.tile([128, OSL[0]], BF16, tag=f"tj{j}",
                                    name=f"tj{j}") for j in range(NB)]
                    for j in range(NB):
                        nc.scalar.activation(
                            out=tj[j][:, 0:w], in_=yv[:, 1, :, j],
                            func=mybir.ActivationFunctionType.Copy,
                            scale=coefc[:, 1, j:j + 1],
                        )
                    t01 = work.tile([128, OSL[0]], BF16, tag="t01")
                    nc.gpsimd.tensor_tensor(out=t01[:, 0:w],
                                            in0=tj[0][:, 0:w],
                                            in1=tj[1][:, 0:w],
                                            op=mybir.AluOpType.add)
                    nc.gpsimd.tensor_tensor(out=out_sb[:, ocols[1]],
                                            in0=t01[:, 0:w],
                                            in1=tj[2][:, 0:w],
                                            op=mybir.AluOpType.add)
                    nc.sync.dma_start(out=out[:, ocols[1]],
                                      in_=out_sb[:, ocols[1]])
                else:
                    # last strip: one fused mult + one reduce across both
                    # batch tiles on DVE, single store — shortest tail
                    cb4 = bass.AP(
                        tensor=coefc.tensor, offset=coefc.offset,
                        ap=[list(coefc.ap[0]), list(coefc.ap[1]), [0, w],
                            list(coefc.ap[2])],
                    )
                    tmp1 = work.tile([128, MT, OSL[1], NB], BF16,
                                     tag="tmp1")
                    nc.vector.tensor_tensor(
                        out=tmp1, in0=yv, in1=cb4,
                        op=mybir.AluOpType.mult,
                    )
                    oall = out_sb[:, OBASE[1]:OBASE[1] + MT * w].rearrange(
                        "p (m o) -> p m o", m=MT)
                    with nc.allow_low_precision("bf16 out, tol 2e-2"):
                        nc.vector.reduce_sum(
                            out=oall, in_=tmp1, axis=mybir.AxisListType.X,
                        )
                    nc.sync.dma_start(
                        out=out[:, OBASE[1]:OBASE[1] + MT * w],
                        in_=out_sb[:, OBASE[1]:OBASE[1] + MT * w])

            # keep the coef chain's PE ops ahead of the bulk of the main
            # sweep so coef is ready when strip 0 finishes accumulating
            add_dep_helper(first_mm[(0, 1, 0)].ins, g2.ins, sync=False,
                           reason="run gather matmuls before strip0 k1")
            add_dep_helper(first_mm[(0, 2, 0)].ins, lmms[-1].ins, sync=False,
                           reason="run logit matmuls before strip0 k2")
            add_dep_helper(first_mm[(1, 0, 0)].ins, lmms[-1].ins, sync=False,
                           reason="run logit matmuls before strip1")

    nc.compile()
    return nc


def _get_nc():
    global _NC_CACHE
    if _NC_CACHE is None:
        _NC_CACHE = _build_nc()
    return _NC_CACHE


def _make_in_maps(x, idx_author, idx_citation, emb_author, emb_citation,
                  W1, b1, W2, W3, b3):
    f = np.float32
    bf = ml_dtypes.bfloat16
    x = np.asarray(x, dtype=f)
    W3r = np.asarray(W3, dtype=f).reshape(IN, OUT, NB)
    b3r = np.asarray(b3, dtype=f).reshape(IN, OUT)
    W1 = np.asarray(W1, dtype=f)
    b1 = np.asarray(b1, dtype=f)

    # header: gather tables G = emb @ W1half.T (+ b1/2 each), W2.T, and the
    # per-core idx/iota rows (filled per shard below)
    hdr = np.zeros((128, HC), f)
    hdr[:VOCAB, :KQ] = np.asarray(emb_author, dtype=f) @ W1[:, :EMB].T \
        + 0.5 * b1
    hdr[:VOCAB, KQ:G2] = np.asarray(emb_citation, dtype=f) @ W1[:, EMB:].T \
        + 0.5 * b1
    hdr[:KQ, G2:G2 + NB] = np.asarray(W2, dtype=f).T
    hdr[IR0 + 1, IC0:IC0 + 2 * BS] = 1.0
    hdr[IR0, IC0 + 2 * BS:] = 1.0
    hdr[IR0 + 1, IC0 + 2 * BS:] = -np.arange(VOCAB, dtype=f)
    hdr = hdr.astype(bf)

    ia = np.asarray(idx_author).astype(bf)
    ic = np.asarray(idx_citation).astype(bf)

    # per out-shard weight strips, bias folded in, ob-major, k-packed,
    # j innermost: [128, sum_ob KT*OSL[ob]*NB]
    shw = OUT // Q_O
    wc_blocks = []
    for oj in range(Q_O):
        cols = slice(oj * shw, (oj + 1) * shw)
        blk = (W3r[:, cols, :] + b3r[:, cols, None]).astype(bf)  # [IN,256,NB]
        strips = []
        for ob in range(OB):
            w = OSL[ob]
            sub = blk[:, OOFF[ob]:OOFF[ob] + w, :].reshape(IN, w * NB)
            strips.append(sub.reshape(KT, 128, w * NB).transpose(1, 0, 2)
                          .reshape(128, KT * w * NB))
        wc_blocks.append(np.ascontiguousarray(
            np.concatenate(strips, axis=1)))

    # x.T per batch shard, k packed: [128, KT*BS]
    xb = x.astype(bf)
    xt_shards = []
    for bi in range(P_B):
        xs = xb[bi * BS:(bi + 1) * BS, :].T               # [IN, BS]
        xs = xs.reshape(KT, 128, BS).transpose(1, 0, 2)
        xt_shards.append(np.ascontiguousarray(xs.reshape(128, KT * BS)))

    in_maps = []
    for c in range(P_B * Q_O):
        bi, oj = c // Q_O, c % Q_O  # 4 batch shards x 2 out shards
        h = hdr.copy()
        h[IR0, IC0:IC0 + BS] = ia[bi * BS:(bi + 1) * BS]
        h[IR0, IC0 + BS:IC0 + 2 * BS] = ic[bi * BS:(bi + 1) * BS]
        in_maps.append({
            "hdr": np.ascontiguousarray(h),
            "xt": xt_shards[bi],
            "wc": wc_blocks[oj],
        })
    return in_maps


def kernel(x, idx_author, idx_citation, emb_author, emb_citation,
           W1, b1, W2, W3, b3):
    global LAST_RESULT
    _ensure_ntff_hook_module()
    nc = _get_nc()
    in_maps = _make_in_maps(x, idx_author, idx_citation, emb_author,
                            emb_citation, W1, b1, W2, W3, b3)
    res = run_bass_kernel_spmd(nc, in_maps, core_ids=list(range(P_B * Q_O)))
    LAST_RESULT = res
    outa = np.empty((B, OUT), dtype=np.float32)
    shw = OUT // Q_O
    for c in range(P_B * Q_O):
        bi, oj = c // Q_O, c % Q_O
        blk = res.results[c]["out"].astype(np.float32)   # [128, MT*(160+96)]
        for ob in range(OB):
            w = OSL[ob]
            sub = blk[:, OBASE[ob]:OBASE[ob] + MT * w].reshape(128, MT, w)
            outa[bi * BS:(bi + 1) * BS,
                 oj * shw + OOFF[ob]:oj * shw + OOFF[ob] + w] = \
                sub.transpose(1, 0, 2).reshape(BS, w)
    return outa

